# revision 39
# baseline (speedup 1.0000x reference)
"""Trainium2 Bass kernel for nn_Axial_PFCU_Continuous (dense_cnn).

All linear terms ride the PE:
  z = W0 @ x  (bf16; W0 = GAMMA*(Wf~ diag(c0) + diag(cB0)))
      + 16 shift terms (mixer taps at +-4/8/16 on H and W, edge taps at +-1)
        packed as 8 fp8 DoubleRow matmuls, two terms per matmul: the rhs is a
        zero-padded fp8 image and the DoubleRow "t" dimension is given an
        arbitrary 2-D displacement stride, so each matmul contracts two
        different shifted windows of the same buffer (cost: 0.5 cyc/pixel
        per matmul, contraction-independent).
  pre = PReLU(z/16 + bz) on Act; coord-attention stats on DVE/Pool;
  row gates (ah) applied as groups of rows complete, column gate (aw) and
  output DMA drain in the tail.

Sharding: pure data-parallel, 1 of 8 batch samples per NeuronCore.
"""
import sys
import math

sys.path.insert(0, '/opt/trn_rl_repo')

import numpy as np
import ml_dtypes
from contextlib import ExitStack

import concourse.bass as bass
import concourse.bacc as bacc
from concourse import mybir, tile
from concourse.bass_utils import run_bass_kernel_spmd
from concourse.ap import AP as APC

f32 = mybir.dt.float32
bf16 = mybir.dt.bfloat16
fp8 = mybir.dt.float8e4
ALU = mybir.AluOpType
AF = mybir.ActivationFunctionType
PM = mybir.MatmulPerfMode
AX = mybir.AxisListType

B, C, H, W = 8, 96, 128, 128
HW = H * W
PAD = 16
HP, WP = H + 2 * PAD, W + 2 * PAD      # padded fp8 image dims (160 x 160)
EPS = 1e-5
GAMMA = 16.0
BH = 8                                 # rows per block
NB = H // BH                           # 16 blocks
CH = 4                                 # psum chunk rows (one bank)
N_CORES = 8
# ah groups: rows gated once the closing block's xh stats are in
GROUPS = {3: (0, 32), 7: (32, 64), 11: (64, 96), 13: (96, 112),
          14: (112, 120)}

_GRAPH_CACHE = {}


# ----------------------------------------------------------------- host folds
def _taps(w_taps, r):
    r = max(float(r), 1.0)
    K = w_taps.shape[1]
    d2w = {}
    for i in range(K):
        s = (i - K // 2) * r
        f = math.floor(s)
        frac = s - f
        for d, wt in ((int(f), 1.0 - frac), (int(f) + 1, frac)):
            if wt != 0.0:
                if d not in d2w:
                    d2w[d] = np.zeros(C, np.float64)
                d2w[d] = d2w[d] + wt * np.asarray(w_taps[:, i], np.float64)
    return {d: w for d, w in d2w.items() if abs(d) < H}


def _merge(a, b):
    out = dict(a)
    for d, w in b.items():
        out[d] = out.get(d, np.zeros(C, np.float64)) + w
    return out


class _Pack:
    def __init__(self, rows):
        self.rows = rows
        self.cols = {}
        self.parts = []
        self.pos = 0

    def put(self, name, arr):
        arr = np.asarray(arr, np.float64)
        if arr.ndim == 1:
            arr = arr[:, None]
        pad = np.zeros((self.rows, arr.shape[1]), np.float64)
        pad[:arr.shape[0], :] = arr
        self.cols[name] = (self.pos, arr.shape[1])
        self.parts.append(pad)
        self.pos += arr.shape[1]

    def done(self, dt):
        return np.concatenate(self.parts, axis=1).astype(dt)


def _fold(inp):
    g = lambda k: np.asarray(inp[k], np.float64)
    hA = _merge(_taps(g('wh_m'), float(np.asarray(inp['r_m']))),
                _taps(g('wh_l'), float(np.asarray(inp['r_l']))))
    wA = _merge(_taps(g('ww_m'), float(np.asarray(inp['r_m']))),
                _taps(g('ww_l'), float(np.asarray(inp['r_l']))))
    hA[0] = hA.get(0, np.zeros(C)) + 2.0    # identity terms of m+l
    wA.setdefault(0, np.zeros(C))
    c0 = hA[0] + wA[0]

    sf = g('bnf_g') / np.sqrt(g('bnf_v') + EPS)
    wf = g('w_fuse') * sf[:, None]            # (Cout, Cin) BN-folded
    bf_ = g('bnf_b') - g('bnf_m') * sf

    ds = g('dg_g') / np.sqrt(g('dg_v') + EPS)
    db = g('dg_b') - g('dg_m') * ds
    dg_wh, dg_ww = g('dg_wh'), g('dg_ww')
    ehm1, eh0, ehp1 = ds * dg_wh[:, 0], ds * (dg_wh[:, 1] + 1.0), ds * dg_wh[:, 2]
    ewm1, ew0, ewp1 = ds * dg_ww[:, 0], ds * dg_ww[:, 1], ds * dg_ww[:, 2]
    cB0 = eh0 + ew0
    bz = bf_ + db

    cs = g('ca_g') / np.sqrt(g('ca_v') + EPS)
    cb = g('ca_b') - g('ca_m') * cs

    # fp8 PE terms: (dr, dc, (Cout, Cin) matrix), all GAMMA-scaled
    terms = []
    for d in sorted(hA):
        if d != 0:
            assert abs(d) <= PAD, d
            terms.append((d, 0, GAMMA * wf * hA[d][None, :]))
    for d in sorted(wA):
        if d != 0:
            assert abs(d) <= PAD, d
            terms.append((0, d, GAMMA * wf * wA[d][None, :]))
    terms.append((-1, 0, np.diag(GAMMA * ehm1)))
    terms.append((1, 0, np.diag(GAMMA * ehp1)))
    terms.append((0, -1, np.diag(GAMMA * ewm1)))
    terms.append((0, 1, np.diag(GAMMA * ewp1)))
    if len(terms) % 2:
        terms.append((0, 0, np.zeros((C, C))))
    terms.sort(key=lambda t: t[0] * WP + t[1])

    pairs = []
    pkq = _Pack(C)
    for i in range(0, len(terms), 2):
        r0_, c0_, A0 = terms[i]
        r1_, c1_, A1 = terms[i + 1]
        s_t = (r1_ - r0_) * WP + (c1_ - c0_)
        assert s_t > 0, (terms[i][:2], terms[i + 1][:2])
        pairs.append(((r0_, c0_), s_t))
        pkq.put(f'P{i // 2}', np.concatenate([A0.T, A1.T], axis=1))
    constq = pkq.done(ml_dtypes.float8_e4m3)

    pkb = _Pack(C)
    W0 = GAMMA * (wf * c0[None, :] + np.diag(cB0))
    pkb.put('W0T', W0.T)
    pkb.put('caw1_tb', (g('ca_w1') / float(W)).T)     # (C, 8)
    pkb.put('caww_tb', g('ca_ww').T)                  # (8, C)
    constb = pkb.done(ml_dtypes.bfloat16)

    pkf = _Pack(C)
    pkf.put('bz', bz)
    pkf.put('act_a', g('act_a'))
    pkf.put('zero', np.zeros(C))
    pkf.put('caw1_t', (g('ca_w1') / float(W)).T)      # (C, 8) f32
    pkf.put('cas', cs)
    pkf.put('cab', cb)
    pkf.put('caa', g('ca_a'))
    pkf.put('cawh_t', g('ca_wh').T)                   # (8, C)
    consts = pkf.done(np.float32)

    key = (tuple(pairs), consts.shape[1], constb.shape[1], constq.shape[1])
    return consts, pkf.cols, constb, pkb.cols, constq, pkq.cols, pairs, key


# -------------------------------------------------------------- graph builder
def _build(pairs, colf, colb, colq, ckf, ckb, ckq):
    nc = bacc.Bacc()
    xb_p = nc.declare_dram_parameter("xb", (C, HW), bf16, isOutput=False)
    xp_p = nc.declare_dram_parameter("xpad", (C, HP * WP), fp8, isOutput=False)
    cf_p = nc.declare_dram_parameter("consts", (C, ckf), f32, isOutput=False)
    cb_p = nc.declare_dram_parameter("constb", (C, ckb), bf16, isOutput=False)
    cq_p = nc.declare_dram_parameter("constq", (C, ckq), fp8, isOutput=False)
    o_p = nc.declare_dram_parameter("out", (C, HW), bf16, isOutput=True)

    with tile.TileContext(nc) as tc, ExitStack() as ctx:
        big = ctx.enter_context(tc.tile_pool(name="big", bufs=1))
        f1p = ctx.enter_context(tc.tile_pool(name="f1p", bufs=4))
        f2p = ctx.enter_context(tc.tile_pool(name="f2p", bufs=4))
        awf = ctx.enter_context(tc.tile_pool(name="awf", bufs=2))
        y2p = ctx.enter_context(tc.tile_pool(name="y2p", bufs=2))
        psq = ctx.enter_context(tc.tile_pool(name="psq", bufs=3, space="PSUM"))
        psw = ctx.enter_context(tc.tile_pool(name="psw", bufs=1, space="PSUM"))
        pss = ctx.enter_context(tc.tile_pool(name="pss", bufs=1, space="PSUM"))

        cst = big.tile([C, ckf], f32, tag="cst")
        cbt = big.tile([C, ckb], bf16, tag="cbt")
        cqt = big.tile([C, ckq], fp8, tag="cqt")

        def cc(name):
            p0, _ = colf[name]
            return cst[:, p0:p0 + 1]

        def crf(name, rows=C):
            p0, n = colf[name]
            return cst[0:rows, p0:p0 + n]

        def cbr(name, rows=C):
            p0, n = colb[name]
            return cbt[0:rows, p0:p0 + n]

        def cq(i):
            p0, n = colq[f'P{i}']
            return cqt[0:C, p0:p0 + n].rearrange("p (t m) -> p t m", t=2)

        xb_sb = big.tile([C, HW], bf16, tag="xb")
        xp_sb = big.tile([C, HP * WP], fp8, tag="xpad")
        pre = big.tile([C, HW], bf16, tag="pre")
        scr = big.tile([C, 512], bf16, tag="scr")
        xwacE = big.tile([C, BH * W], bf16, tag="xwacE")
        xwacO = big.tile([C, BH * W], bf16, tag="xwacO")
        yin = big.tile([C, H], f32, tag="yin")
        ah = big.tile([C, H], bf16, tag="ah")
        aw = big.tile([C, W], bf16, tag="aw")
        xw = big.tile([C, W], bf16, tag="xw")

        xb3 = xb_sb[:].rearrange("p (h w) -> p h w", w=W)
        pre3 = pre[:].rearrange("p (h w) -> p h w", w=W)
        xwacE3 = xwacE[:].rearrange("p (h w) -> p h w", w=W)
        xwacO3 = xwacO[:].rearrange("p (h w) -> p h w", w=W)
        o3 = o_p[:].rearrange("p (h w) -> p h w", w=W)
        xp_t = xp_sb[:].tensor
        zcol = cc('zero')

        def xpdma(eng, a, b):
            eng.dma_start(xp_sb[:, a * WP:b * WP], xp_p[:, a * WP:b * WP])

        def xbdma(eng, a, b):
            eng.dma_start(xb_sb[:, a * W:b * W], xb_p[:, a * W:b * W])

        # ---- input DMA (3 queues, first-needed first) ----
        xbdma(nc.scalar, 0, 8)
        nc.scalar.dma_start(cbt[:], cb_p[:])
        nc.scalar.dma_start(cqt[:], cq_p[:])
        nc.scalar.dma_start(cst[:], cf_p[:])
        xpdma(nc.sync, 0, 24)
        xbdma(nc.sync, 8, 24)
        xbdma(nc.sync, 24, 48)
        xbdma(nc.sync, 48, 80)
        xbdma(nc.sync, 80, 128)
        nc.gpsimd.memset(scr[:], 0.0)
        xpdma(nc.gpsimd, 24, 56)
        xpdma(nc.gpsimd, 56, 88)
        xpdma(nc.gpsimd, 88, 124)
        xpdma(nc.gpsimd, 124, 160)

        # ---- PE p-state + act-table warmups ----
        for i in range(4):
            pw = psw.tile([C, 512], f32, tag="warm")
            nc.tensor.matmul(pw[:], scr[:, 0:96], scr[:], start=True, stop=True)
        nc.scalar.activation(scr[:, 0:1], zcol, AF.Prelu, bias=zcol,
                             scale=1.0, alpha=cc('act_a'))
        nc.scalar.activation(scr[:, 1:2], zcol, AF.Sigmoid, bias=zcol,
                             scale=1.0)

        def qrhs(r0, rows, pair):
            (dr, dc), s_t = pair
            off = (r0 + PAD + dr) * WP + (PAD + dc)
            return APC(xp_t, off, [[HP * WP, C], [s_t, 2], [WP, rows], [1, W]])

        def do_block(b):
            r0, r1 = b * BH, (b + 1) * BH
            pk = psq.tile([C, BH, W], f32, tag="pk")
            for k0 in (r0, r0 + CH):
                out = pk[:, k0 - r0:k0 - r0 + CH, :]
                nc.tensor.matmul(out, cbr('W0T'), xb3[:, k0:k0 + CH, :],
                                 start=True, stop=False)
                for i, pr in enumerate(pairs):
                    nc.tensor.matmul(out, cq(i), qrhs(k0, CH, pr),
                                     start=False, stop=(i == len(pairs) - 1),
                                     perf_mode=PM.DoubleRow)
            xwac3 = xwacE3 if b % 2 == 0 else xwacO3
            xw_eng = nc.gpsimd if b % 2 == 0 else nc.vector
            if b == NB - 1:
                # last block: per-chunk PReLU + split accumulator add so the
                # aw chain starts as soon as the first half drains
                nc.scalar.activation(pre3[:, r0:r0 + CH, :], pk[:, 0:CH, :],
                                     AF.Prelu, bias=cc('bz'),
                                     scale=1.0 / GAMMA, alpha=cc('act_a'))
                xw_eng.tensor_tensor(xwac3[:, 0:CH, :], xwac3[:, 0:CH, :],
                                     pre3[:, r0:r0 + CH, :], op=ALU.add)
                nc.scalar.activation(pre3[:, r0 + CH:r1, :], pk[:, CH:BH, :],
                                     AF.Prelu, bias=cc('bz'),
                                     scale=1.0 / GAMMA, alpha=cc('act_a'))
                xw_eng.tensor_tensor(xwac3[:, CH:BH, :], xwac3[:, CH:BH, :],
                                     pre3[:, r0 + CH:r1, :], op=ALU.add)
            else:
                nc.scalar.activation(pre3[:, r0:r1, :], pk[:], AF.Prelu,
                                     bias=cc('bz'), scale=1.0 / GAMMA,
                                     alpha=cc('act_a'))
                if b < 2:
                    xw_eng.tensor_copy(xwac3, pre3[:, r0:r1, :])
                else:
                    xw_eng.tensor_tensor(xwac3, xwac3, pre3[:, r0:r1, :],
                                         op=ALU.add)
            f_eng = nc.gpsimd if b == NB - 1 else nc.vector
            f1 = f1p.tile([C, BH * (W // 2)], bf16, tag="f1")
            f13 = f1[:].rearrange("p (h w) -> p h w", w=W // 2)
            f_eng.tensor_tensor(f13, pre3[:, r0:r1, 0:W // 2],
                                pre3[:, r0:r1, W // 2:W], op=ALU.add)
            f2 = f2p.tile([C, BH * (W // 4)], bf16, tag="f2")
            f23 = f2[:].rearrange("p (h w) -> p h w", w=W // 4)
            f_eng.tensor_tensor(f23, f13[:, :, 0:W // 4],
                                f13[:, :, W // 4:W // 2], op=ALU.add)
            nc.vector.tensor_reduce(yin[:, r0:r1], f23, axis=AX.X, op=ALU.add)

        def ca_group(g0, g1):
            y1 = pss.tile([C, 512], f32, tag="small")
            nc.tensor.matmul(y1[0:8, 0:g1 - g0], crf('caw1_t'), yin[:, g0:g1],
                             start=True, stop=True)
            y2 = y2p.tile([8, g1 - g0], f32, tag="y2g")
            nc.scalar.activation(y2[:], y1[0:8, 0:g1 - g0], AF.Prelu,
                                 bias=cc('cab')[0:8, :], scale=cc('cas')[0:8, :],
                                 alpha=cc('caa')[0:8, :])
            ahg = pss.tile([C, 512], f32, tag="small")
            nc.tensor.matmul(ahg[0:C, 0:g1 - g0], crf('cawh_t', rows=8), y2[:],
                             start=True, stop=True)
            nc.scalar.activation(ah[:, g0:g1], ahg[0:C, 0:g1 - g0], AF.Sigmoid,
                                 bias=zcol, scale=1.0)

        def ah_gate(j, eng=None):
            r0, r1 = j * BH, (j + 1) * BH
            ah_b = ah[:, r0:r1].unsqueeze(2).broadcast_to((C, BH, W))
            (eng or nc.gpsimd).tensor_tensor(pre3[:, r0:r1, :],
                                             pre3[:, r0:r1, :], ah_b,
                                             op=ALU.mult)

        def aw_gate(r0, r1, eng):
            aw_b = aw[:].unsqueeze(1).broadcast_to((C, r1 - r0, W))
            eng.tensor_tensor(pre3[:, r0:r1, :], pre3[:, r0:r1, :], aw_b,
                              op=ALU.mult)

        def out_dma(r0, r1, eng):
            eng.dma_start(o3[:, r0:r1, :], pre3[:, r0:r1, :])

        # ---- main stream; group closes emitted one block late so the
        # next block's PReLU/stats stay at its engines' queue heads ----
        xwE = awf.tile([C, 2 * W], bf16, tag="awtE2")
        for b in range(NB):
            do_block(b)
            if b == NB - 1:
                # even accumulator closed at block 14: pre-fold during b15
                tE = awf.tile([C, 4 * W], bf16, tag="awtE")
                tE3 = tE[:].rearrange("p (h w) -> p h w", w=W)
                nc.gpsimd.tensor_tensor(tE3, xwacE3[:, 0:4, :],
                                        xwacE3[:, 4:8, :], op=ALU.add)
                nc.gpsimd.tensor_tensor(xwE[:, 0:W], tE[:, 0:W],
                                        tE[:, W:2 * W], op=ALU.add)
                nc.gpsimd.tensor_tensor(xwE[:, W:2 * W], tE[:, 2 * W:3 * W],
                                        tE[:, 3 * W:4 * W], op=ALU.add)
                nc.gpsimd.tensor_tensor(xwE[:, 0:W], xwE[:, 0:W],
                                        xwE[:, W:2 * W], op=ALU.add)
            gb = b - 1
            if gb in GROUPS:
                g0, g1 = GROUPS[gb]
                ca_group(g0, g1)
                for j in range(g0 // BH, (g1 + BH - 1) // BH):
                    ah_gate(j)
        # ---- tail: odd-side folds + aw chain + last ah group ----
        tO = awf.tile([C, 4 * W], bf16, tag="awtO")
        tO3 = tO[:].rearrange("p (h w) -> p h w", w=W)
        nc.vector.tensor_tensor(tO3, xwacO3[:, 0:4, :], xwacO3[:, 4:8, :],
                                op=ALU.add)
        nc.vector.tensor_tensor(tO[:, 0:W], tO[:, 0:W], tO[:, W:2 * W],
                                op=ALU.add)
        nc.vector.tensor_tensor(tO[:, W:2 * W], tO[:, 2 * W:3 * W],
                                tO[:, 3 * W:4 * W], op=ALU.add)
        nc.vector.tensor_tensor(tO[:, 0:W], tO[:, 0:W], tO[:, W:2 * W],
                                op=ALU.add)
        nc.vector.tensor_tensor(xw[:], xwE[:, 0:W], tO[:, 0:W], op=ALU.add)
        y1w = pss.tile([C, 512], f32, tag="small")
        nc.tensor.matmul(y1w[0:8, 0:W], cbr('caw1_tb'), xw[:],
                         start=True, stop=True)
        y2w = y2p.tile([8, W], bf16, tag="y2w")
        nc.scalar.activation(y2w[:], y1w[0:8, 0:W], AF.Prelu,
                             bias=cc('cab')[0:8, :], scale=cc('cas')[0:8, :],
                             alpha=cc('caa')[0:8, :])
        awp = pss.tile([C, 512], f32, tag="small")
        nc.tensor.matmul(awp[0:C, 0:W], cbr('caww_tb', rows=8), y2w[:],
                         start=True, stop=True)
        nc.scalar.activation(aw[:], awp[0:C, 0:W], AF.Sigmoid, bias=zcol,
                             scale=1.0)
        ca_group(120, 128)
        ah_gate(15)
        dq = [nc.sync, nc.scalar, nc.sync, nc.scalar, nc.sync, nc.scalar,
              nc.sync, nc.gpsimd]
        for si, s0 in enumerate(range(0, 128, 16)):
            geng = nc.vector if si in (0, 2, 4, 6, 7) else nc.gpsimd
            aw_gate(s0, s0 + 16, geng)
            out_dma(s0, s0 + 16, dq[si])

    nc.compile()
    return nc


def _get_graph(key, pairs, colf, colb, colq, ckf, ckb, ckq):
    if key not in _GRAPH_CACHE:
        _GRAPH_CACHE[key] = _build(pairs, colf, colb, colq, ckf, ckb, ckq)
    return _GRAPH_CACHE[key]


# ------------------------------------------------------------------ interface
def _run(inputs, trace=False):
    x = np.ascontiguousarray(np.asarray(inputs['x'], np.float32))
    assert x.shape == (B, C, H, W)
    (consts, colf, constb, colb, constq, colq, pairs, key) = _fold(inputs)
    nc = _get_graph(key, pairs, colf, colb, colq,
                    consts.shape[1], constb.shape[1], constq.shape[1])
    xb = x.astype(ml_dtypes.bfloat16).reshape(B, C, HW)
    xpad = np.zeros((B, C, HP, WP), ml_dtypes.float8_e4m3)
    xpad[:, :, PAD:PAD + H, PAD:PAD + W] = x.astype(ml_dtypes.float8_e4m3)
    in_maps = []
    for i in range(N_CORES):
        in_maps.append({'xb': xb[i].copy(),
                        'xpad': xpad[i].reshape(C, HP * WP).copy(),
                        'consts': consts, 'constb': constb, 'constq': constq})
    res = run_bass_kernel_spmd(nc, in_maps, list(range(N_CORES)), trace=trace)
    outs = [res.results[i]['out'].astype(np.float32).reshape(C, H, W)
            for i in range(N_CORES)]
    return np.stack(outs, axis=0), res


def kernel(**inputs):
    out, _ = _run(inputs, trace=False)
    return out
